# revision 1
# baseline (speedup 1.0000x reference)
"""Trainium2 Bass kernel for a ResNet BasicBlock (dense CNN, sync-BN).

Reference computation (training-mode BN, batch stats over (N,H,W)):
    h = conv3x3(x, W1) * mask1            # structured channel pruning
    h = relu(bn(h, gamma1, beta1))
    h = conv3x3(h, W2) * mask2
    h = bn(h, gamma2, beta2)
    out = relu(h + x)                      # identity shortcut

Shapes: x [32, 256, 56, 56] f32, W [256, 256, 3, 3] f32.

Strategy: data-parallel over batch N across 8 NeuronCores (4 images per
core), weights replicated.  BN batch statistics are synchronized with a
tiny AllReduce of per-channel (sum, sum-of-squares) pairs whose DMA
descriptors are pre-generated at kernel start and fired by a single
gpsimd trigger when the stats are ready.

Channel sparsity: mask1 zeroes ~half of conv1's output channels, and
(when beta1 <= 0 on those channels, which holds for the reference
inputs) the corresponding h1 channels are exactly zero, so conv1 only
computes the a1=|mask1| active channels and conv2 only consumes them.
Active channels are packed into partition groups of <=128 via a
host-side permutation folded into the weights:
  - conv1 output groups: [128, a1-128]; the small overflow group's
    conv2 contribution is evaluated as ONE im2col matmul per chunk
    (K = 9 taps * (a1-128) channels <= 128) against a plane holding 9
    tap-shifted replicas of the overflow channels (built by SBUF->SBUF
    DMAs; the center tap block sits at partition 0 so compute APs stay
    partition-aligned, and the block order is folded into the weights).
  - conv2 output stays in TRUE channel layout (inactive mask2 rows are
    zeroed in the weights), so the residual tail needs no scatter.

BN statistics are produced by the DVE bn_stats/bn_aggr instructions
(one Welford 6-tuple per PSUM chunk), so the ACT engine only does the
PSUM->SBUF drain copies; ACT traffic is what contends with the PE's
SBUF rhs stream, so keeping it minimal preserves matmul pitch.

Per-core layout: zero-padded 58x58 bf16 planes so each conv tap is an
offset shift; 7 chunks of 8 output rows per image so a chunk fits a
PSUM bank.  Head loads / BN1 applies are emitted interleaved with the
conv work (one image of lookahead) so the strict-FIFO ACT/DVE queues
never dam up behind bulk phases.
"""

import numpy as np
import ml_dtypes

# ---- problem constants (hardcoded; kernel.py must be self-contained) ----
N_TOT, C, H, W = 32, 256, 56, 56
N_CORES = 8
NL = N_TOT // N_CORES          # images per core
PW = H + 2                     # padded row stride (58)
PLANE = PW * PW + 4            # padded plane floats + 4 spare for tap overreads
STRIP0 = PW + 1                # first interior output position (59)
CHUNK = 8 * PW                 # 464: 8 output rows per chunk
NCHUNK = 7                     # 7 chunks * 8 rows = 56 rows
HW = H * W                     # 3136
HALF_ROWS = 28                 # row granularity for x/out streaming DMAs
HALF_ELEMS = HALF_ROWS * W     # 1568
COUNT = N_TOT * HW             # sync-BN element count per channel
CL = NL * HW                   # local per-core element count per channel
EPS = 1e-5

_BF16 = ml_dtypes.bfloat16

_cache = {}

TAPS = [(ky, kx) for ky in range(3) for kx in range(3)]
BORD = [4, 0, 1, 2, 3, 5, 6, 7, 8]   # i2c plane tap-block order, center first


def _make_plan(mask1, beta1):
    """Conv1 output channel groups (== conv2 input groups), true-channel ids."""
    act1 = np.flatnonzero(mask1 != 0)
    inact = np.flatnonzero(mask1 == 0)
    if np.any(np.maximum(beta1[inact], 0.0) != 0.0) or len(act1) == 0:
        act1 = np.arange(C)    # dense fallback: every channel treated live
    groups = [act1[i:i + 128] for i in range(0, len(act1), 128)]
    kinds = ["i2c" if (9 * len(g) <= 128 and len(g) < 128) else "full"
             for g in groups]
    return groups, kinds


def _pack_weights(W1, W2, mask2, groups, kinds):
    m2 = mask2.reshape(2, 128).astype(np.float32)
    packs = {}
    for gi, (g, kind) in enumerate(zip(groups, kinds)):
        s = len(g)
        blocks = []
        for h in range(2):
            for (ky, kx) in TAPS:
                blocks.append(W1[g, 128 * h:128 * h + 128, ky, kx].T)  # [ci,co]
        packs[f"wt1_{gi}"] = np.ascontiguousarray(
            np.concatenate(blocks, axis=1)).astype(_BF16)      # [128, 18*s]
        if kind == "full":
            blocks = []
            for j in range(2):
                for (ky, kx) in TAPS:
                    blk = (W2[128 * j:128 * j + 128, :, ky, kx][:, g]
                           * m2[j][:, None]).T                  # [s, 128co]
                    blocks.append(blk)
            packs[f"wt2m_{gi}"] = np.ascontiguousarray(
                np.concatenate(blocks, axis=1)).astype(_BF16)   # [s, 18*128]
        else:
            # block order: center tap first so the plane's compute-written
            # rows sit at partition base 0 (engine AP alignment rule)
            arr = np.zeros((9 * s, 256), np.float32)
            for b, t in enumerate(BORD):
                ky, kx = TAPS[t]
                for j in range(2):
                    arr[b * s:(b + 1) * s, j * 128:(j + 1) * 128] = \
                        (W2[128 * j:128 * j + 128, :, ky, kx][:, g]
                         * m2[j][:, None]).T
            packs[f"wt2o_{gi}"] = arr.astype(_BF16)             # [9s, 256]
    return packs


def _pack_aff(gamma1, beta1, gamma2, beta2, groups):
    G = len(groups)
    out = np.zeros((128, 2 * G + 4), np.float32)
    for gi, g in enumerate(groups):
        s = len(g)
        reps = 9 if (9 * s <= 128 and s < 128) else 1
        out[:s * reps, gi] = np.tile(gamma1[g], reps)
        out[:s * reps, G + gi] = np.tile(beta1[g], reps)
    g2 = np.asarray(gamma2, np.float32).reshape(2, 128)
    b2 = np.asarray(beta2, np.float32).reshape(2, 128)
    out[:, 2 * G + 0] = g2[0]
    out[:, 2 * G + 1] = g2[1]
    out[:, 2 * G + 2] = b2[0]
    out[:, 2 * G + 3] = b2[1]
    return out


def _build(groups, kinds):
    import concourse.bass as bass_mod
    import concourse.bacc as bacc
    import concourse.mybir as mybir
    import concourse.tile as tile

    f32 = mybir.dt.float32
    bf16 = mybir.dt.bfloat16
    AX = mybir.AxisListType
    ALU = mybir.AluOpType
    AF = mybir.ActivationFunctionType

    G = len(groups)
    sizes = [len(g) for g in groups]
    SW = 2 * G                       # stats-exchange width (sums | sumsqs)

    nc = bacc.Bacc("TRN2", target_bir_lowering=False, debug=False,
                   num_devices=N_CORES)

    x_d = nc.dram_tensor("x", [NL, C, H, W], f32, kind="ExternalInput")
    wt1_d = [nc.dram_tensor(f"wt1_{gi}", [128, 18 * sizes[gi]], bf16,
                            kind="ExternalInput") for gi in range(G)]
    wt2_d = []
    for gi in range(G):
        if kinds[gi] == "full":
            wt2_d.append(nc.dram_tensor(f"wt2m_{gi}", [sizes[gi], 18 * 128],
                                        bf16, kind="ExternalInput"))
        else:
            wt2_d.append(nc.dram_tensor(f"wt2o_{gi}", [9 * sizes[gi], 256],
                                        bf16, kind="ExternalInput"))
    aff_d = nc.dram_tensor("aff", [128, SW + 4], f32, kind="ExternalInput")
    out_d = nc.dram_tensor("out", [NL, C, H, W], f32, kind="ExternalOutput")

    replica_groups = [list(range(N_CORES))]

    def interior(tile_ap, base, nrows):
        """[p, nrows, 56] strided view (row stride PW) starting at `base`."""
        v = tile_ap[:, base:base + nrows * PW].rearrange(
            "p (r c) -> p r c", c=PW)
        return v[:, :, 0:W]

    with tile.TileContext(nc) as tc:
        import contextlib
        with contextlib.ExitStack() as ctx:
            const = ctx.enter_context(tc.tile_pool(name="const", bufs=1))
            psum = ctx.enter_context(tc.tile_pool(name="psum", bufs=6,
                                                  space="PSUM"))
            psumb = ctx.enter_context(tc.tile_pool(name="psumb", bufs=2,
                                                   space="PSUM"))
            xst = ctx.enter_context(tc.tile_pool(name="xst", bufs=3))
            otp = ctx.enter_context(tc.tile_pool(name="otp", bufs=2))
            sqp = ctx.enter_context(tc.tile_pool(name="sqp", bufs=2))

            wt1_sb = [const.tile([128, 18 * sizes[gi]], bf16,
                                 tag=f"wt1_{gi}", name=f"wt1_{gi}")
                      for gi in range(G)]
            wt2_sb = []
            for gi in range(G):
                if kinds[gi] == "full":
                    wt2_sb.append(const.tile([sizes[gi], 18 * 128], bf16,
                                             tag=f"wt2_{gi}", name=f"wt2_{gi}"))
                else:
                    wt2_sb.append(const.tile([9 * sizes[gi], 256], bf16,
                                             tag=f"wt2_{gi}", name=f"wt2_{gi}"))
            for gi in range(G):
                nc.sync.dma_start(wt1_sb[gi][:], wt1_d[gi][:])
                nc.sync.dma_start(wt2_sb[gi][:], wt2_d[gi][:])
            aff_sb = const.tile([128, SW + 4], f32, tag="aff", name="aff")
            nc.sync.dma_start(aff_sb[:], aff_d[:])

            # persistent per-image planes
            x_pad = [[const.tile([128, PLANE], bf16, tag=f"xp{j}_{n}",
                                 name=f"xp{j}_{n}")
                      for n in range(NL)] for j in range(2)]
            h1 = []                       # per group: list over images
            for gi in range(G):
                rows = 9 * sizes[gi] if kinds[gi] == "i2c" else sizes[gi]
                h1.append([const.tile([rows, PLANE], bf16, tag=f"h1{gi}_{n}",
                                      name=f"h1{gi}_{n}")
                           for n in range(NL)])
            h2 = [[const.tile([128, HW], bf16, tag=f"h2{j}_{n}",
                              name=f"h2{j}_{n}")
                   for n in range(NL)] for j in range(2)]

            # zero the non-interior positions of padded planes (i2c planes:
            # center block only; other blocks are fully DMA-overwritten with
            # shifted copies whose source pads are these zeros)
            # pad zeroing on gpsimd (idle at entry) so the DVE FIFO is
            # free for the x casts and the first matmul can issue early
            def zero_pads(t, s):
                tt = t[0:s]
                nc.gpsimd.memset(tt[:, 0:STRIP0], 0.0)
                pairs = tt[:, 2 * PW - 1:2 * PW - 1 + 56 * PW].rearrange(
                    "p (r c) -> p r c", c=PW)[:, :, 0:2]
                nc.gpsimd.memset(pairs, 0.0)
                nc.gpsimd.memset(tt[:, STRIP0 + 56 * PW:PLANE], 0.0)

            for j in range(2):
                for n in range(NL):
                    zero_pads(x_pad[j][n], 128)
            for gi in range(G):
                for n in range(NL):
                    zero_pads(h1[gi][n], sizes[gi])


            # ---- cross-core stats exchange plumbing (SBUF remote DMA) ----
            # Two exchanges (BN1, BN2).  Each broadcasts this core's packed
            # [128, SW(=4)] stats tile to all 7 peers with XOR-relative
            # dests; slot d of the receive tile gets the copy from core
            # (me ^ d).  Hardware remote sems count arrivals (2 per
            # transfer -> wait >= 14).  Descriptors are PRE-GENERATED here
            # (they only record addresses); a single gpsimd trigger_dma
            # fires each batch of 7 once the stats tile is written.
            rsem = [nc.alloc_semaphore(f"rst{i}") for i in range(2)]
            lsem = nc.alloc_semaphore("lst")
            _gp_prev = [None]
            deferred_waits = []

            def gp_order(bi):
                if _gp_prev[0] is not None:
                    bass_mod._add_dep_helper(bi.ins, _gp_prev[0].ins,
                                             sync=False,
                                             reason="stats-exchange order")
                _gp_prev[0] = bi
                return bi

            nc._bir_kernel_barrier_sem_replica_groups.extend(
                set(g) for g in replica_groups)

            def defer_wait(bi, sem, val):
                bi._wait_ge(sem, 0)
                deferred_waits.append((bi, sem, val))
                return bi

            # sems persist across NEFF executions: clear them as soon as all
            # cores have entered (peers send >100us later, after conv1)
            for i, s in enumerate(rsem + [lsem]):
                cl = gp_order(nc.gpsimd.sem_clear(s))
                if i == 0:
                    defer_wait(cl, nc._bir_kernel_barrier_sem,
                               nc.bir_kernel_barrier_sem_inc)

            # stats tiles written at conv end; descriptors reference them now
            packed1 = const.tile([128, SW], f32, tag="pk1", name="pk1")
            packed2 = const.tile([128, 4], f32, tag="pk2", name="pk2")
            rv1 = const.tile([128, 8 * SW], f32, tag="rv1", name="rv1")
            rv2 = const.tile([128, 32], f32, tag="rv2", name="rv2")
            for ex, (pk, rv, w) in enumerate(((packed1, rv1, SW),
                                              (packed2, rv2, 4))):
                for d in range(1, 8):
                    rd = [None] * 8
                    rd[d] = (0, d)
                    gp_order(nc.gpsimd.remote_dma_broadcast(
                        rv[:, w * d:w * d + w], pk[:],
                        remote_sem=rsem[ex], local_sem=lsem, rdests=rd))

            # per-(image,chunk) (sum, sumsq) accumulator columns, filled by
            # the ACT drain copies' accum_out and a paired ACT square op
            acc1 = {(gi, sq): const.tile([sizes[gi], NL * NCHUNK], f32,
                                         tag=f"a1{gi}{sq}", name=f"a1{gi}{sq}")
                    for gi in range(G) for sq in ("s", "q")}
            acc2 = {(j, sq): const.tile([128, NL * NCHUNK], f32,
                                        tag=f"a2{j}{sq}", name=f"a2{j}{sq}")
                    for j in range(2) for sq in ("s", "q")}

            # ---- head: stream x in (all DMAs up front; staging ring 4),
            # casts emitted per image inside the conv1 loop below ----
            head_xs = []
            for n in range(NL):
                for rh in range(2):
                    for j in range(2):
                        r0 = rh * HALF_ROWS
                        xs = xst.tile([128, HALF_ELEMS], f32, tag="xs",
                                      name="xs")
                        nc.sync.dma_start(
                            xs[:],
                            x_d[n, j * 128:(j + 1) * 128, r0:r0 + HALF_ROWS, :])
                        head_xs.append((n, rh, j, xs))

            def emit_casts(n):
                for (nn, rh, j, xs) in head_xs:
                    if nn != n:
                        continue
                    r0 = rh * HALF_ROWS
                    dst = interior(x_pad[j][nn], (r0 + 1) * PW + 1, HALF_ROWS)
                    src = xs[:, :].rearrange("p (r c) -> p r c", c=W)
                    nc.vector.tensor_copy(dst, src)

            # ---- conv1: per chunk, one 18-matmul run per output group
            # (runs kept contiguous per PSUM group so the PE's LDWEIGHTS
            # pull-ahead pipelining stays active) ----
            emit_casts(0)
            emit_casts(1)
            c1_last = None
            for n in range(NL):
                if n + 2 < NL:
                    emit_casts(n + 2)
                for k in range(NCHUNK):
                    col = n * NCHUNK + k
                    for gi in range(G):
                        s = sizes[gi]
                        pool = psum if kinds[gi] == "full" else psumb
                        tag = "ps" if kinds[gi] == "full" else "psb"
                        pt = pool.tile([s, 8 * W], f32, tag=tag, name=tag)
                        for idx, (hh, (ky, kx)) in enumerate(
                                (hh, t) for hh in range(2) for t in TAPS):
                            dq = (ky - 1) * PW + (kx - 1)
                            off = STRIP0 + CHUNK * k + dq
                            rhs = x_pad[hh][n][:, off:off + CHUNK].rearrange(
                                "p (r c) -> p r c", c=PW)[:, :, 0:W]
                            nc.tensor.matmul(
                                pt[:], wt1_sb[gi][:, idx * s:(idx + 1) * s],
                                rhs, start=(idx == 0), stop=(idx == 17))
                        base = (1 + 8 * k) * PW + 1
                        dst = interior(h1[gi][n][0:s], base, 8)
                        src = pt[:, 0:8 * W].rearrange("p (r c) -> p r c", c=W)
                        nc.scalar.activation(
                            dst, src, AF.Copy,
                            accum_out=acc1[(gi, "s")][:, col:col + 1])
                        sq = sqp.tile([128, 8 * W], f32, tag="sq", name="sq")
                        c1_last = nc.scalar.activation(
                            sq[0:s, :].rearrange("p (r c) -> p r c", c=W),
                            dst, AF.Square,
                            accum_out=acc1[(gi, "q")][:, col:col + 1])

            # ---- tap-shifted replication of the RAW i2c center block,
            # pinned into the BN1 stats-wait gap (idle DMA fabric) ----
            for gi in range(G):
                if kinds[gi] != "i2c":
                    continue
                s = sizes[gi]
                for n in range(NL):
                    for b, t in enumerate(BORD):
                        if b == 0:
                            continue
                        ky, kx = TAPS[t]
                        dq = (ky - 1) * PW + (kx - 1)
                        dm = nc.sync.dma_start(
                            h1[gi][n][b * s:(b + 1) * s,
                                      STRIP0:STRIP0 + 56 * PW],
                            h1[gi][n][0:s,
                                      STRIP0 + dq:STRIP0 + 56 * PW + dq])
                        bass_mod._add_dep_helper(dm.ins, c1_last.ins,
                                                 sync=True,
                                                 reason="repl in BN1 gap")

            # ---- BN1 stats: aggregate -> (sum, sumsq) -> fire trigger ----
            def emit_stats(accs, accq, packed, base, s):
                nc.vector.tensor_reduce(
                    packed[0:s, base:base + 1], accs[:], axis=AX.X,
                    op=ALU.add)
                return nc.vector.tensor_reduce(
                    packed[0:s, base + 1:base + 2], accq[:], axis=AX.X,
                    op=ALU.add)

            for gi in range(G):
                red1 = emit_stats(acc1[(gi, "s")], acc1[(gi, "q")], packed1,
                                  2 * gi, sizes[gi])
            # replicate i2c stats rows to the 9 tap blocks (idle-gap DMAs) so
            # the exchanged stats -- and hence the BN1 scale/bias produced by
            # the affine -- arrive pre-replicated for the [9s]-row apply
            stats_ready = red1
            for gi in range(G):
                if kinds[gi] != "i2c":
                    continue
                s = sizes[gi]
                cpair = packed1[:, 2 * gi:2 * gi + 2]
                for b in range(1, 9):
                    stats_ready = nc.sync.dma_start(
                        cpair[b * s:(b + 1) * s], cpair[0:s])
            nc.vector.tensor_copy(rv1[:, 0:SW], packed1[:])   # own slot (d=0)
            tr1 = gp_order(nc.gpsimd.trigger_dma(count=7))
            bass_mod._add_dep_helper(tr1.ins, stats_ready.ins, sync=True,
                                     reason="stats1 ready")

            # ---- wait for all 8 contributions, sum slots -> global stats ----
            gl1 = const.tile([128, SW], f32, tag="gl1", name="gl1")
            rec1 = nc.vector.tensor_reduce(
                gl1[:], rv1[:, 0:8 * SW].rearrange("p (s c) -> p c s", c=SW),
                axis=AX.X, op=ALU.add)
            defer_wait(rec1, rsem[0], 14)
            bass_mod._add_dep_helper(rec1.ins, c1_last.ins, sync=True,
                                     reason="recv after conv phase")

            # ---- BN affine from global stats (DVE-only; fast rsqrt) ----
            def bn_affine(gl, w, g_col, b_col, sfx):
                """gl [128, 2w] = (sum, sumsq) pairs -> scale/bias [128, w]."""
                glv = gl[:, 0:2 * w].rearrange("p (g c) -> p g c", c=2)
                mean = const.tile([128, w], f32, tag=f"mean{sfx}",
                                  name=f"mean{sfx}")
                nc.vector.tensor_scalar_mul(mean[:], glv[:, :, 0],
                                            1.0 / COUNT)
                var = const.tile([128, w], f32, tag=f"var{sfx}",
                                 name=f"var{sfx}")
                nc.vector.tensor_tensor(var[:], mean[:], mean[:], ALU.mult)
                nc.vector.scalar_tensor_tensor(
                    var[:], glv[:, :, 1], 1.0 / COUNT, var[:],
                    ALU.mult, ALU.subtract)
                nc.vector.tensor_scalar_add(var[:], var[:], EPS)
                y = const.tile([128, w], f32, tag=f"y{sfx}", name=f"y{sfx}")
                nc.scalar.activation(y[:], var[:], AF.Sqrt)
                nc.vector.reciprocal(y[:], y[:])
                sc = const.tile([128, w], f32, tag=f"sc{sfx}", name=f"sc{sfx}")
                nc.vector.tensor_tensor(sc[:], aff_sb[:, g_col:g_col + w],
                                        y[:], ALU.mult)
                bi = const.tile([128, w], f32, tag=f"bi{sfx}", name=f"bi{sfx}")
                nc.vector.tensor_tensor(bi[:], mean[:], sc[:], ALU.mult)
                nc.vector.tensor_tensor(bi[:], aff_sb[:, b_col:b_col + w],
                                        bi[:], ALU.subtract)
                return sc, bi

            sc1, bi1 = bn_affine(gl1, G, 0, G, "1")

            # ---- per image: BN1 apply (ACT relu, rh-major so conv2's first
            # chunks unblock quickly); i2c groups apply to all 9 replica
            # blocks at once ----
            def emit_apply(n):
                for rh in range(2):
                    base = (1 + rh * HALF_ROWS) * PW + 1
                    for gi in range(G):
                        s = sizes[gi]
                        if kinds[gi] == "i2c":
                            v = interior(h1[gi][n][0:9 * s], base, HALF_ROWS)
                            nc.scalar.activation(
                                v, v, AF.Relu,
                                bias=bi1[0:9 * s, gi:gi + 1],
                                scale=sc1[0:9 * s, gi:gi + 1])
                        else:
                            v = interior(h1[gi][n][0:s], base, HALF_ROWS)
                            nc.scalar.activation(
                                v, v, AF.Relu,
                                bias=bi1[0:s, gi:gi + 1],
                                scale=sc1[0:s, gi:gi + 1])

            c2_last = None
            emit_apply(0)
            for n in range(NL):
                if n + 1 < NL:
                    emit_apply(n + 1)
                for k in range(NCHUNK):
                    col = n * NCHUNK + k
                    for j in range(2):
                        pt = psum.tile([128, 8 * W], f32, tag="ps", name="ps")
                        nmm = sum(9 if kinds[gi] == "full" else 1
                                  for gi in range(G))
                        idx = 0
                        for gi in range(G):
                            s = sizes[gi]
                            if kinds[gi] == "full":
                                for t, (ky, kx) in enumerate(TAPS):
                                    dq = (ky - 1) * PW + (kx - 1)
                                    off = STRIP0 + CHUNK * k + dq
                                    rhs = h1[gi][n][0:s, off:off + CHUNK] \
                                        .rearrange("p (r c) -> p r c",
                                                   c=PW)[:, :, 0:W]
                                    nc.tensor.matmul(
                                        pt[:],
                                        wt2_sb[gi][:, (j * 9 + t) * 128:
                                                   (j * 9 + t + 1) * 128],
                                        rhs, start=(idx == 0),
                                        stop=(idx == nmm - 1))
                                    idx += 1
                            else:
                                off = STRIP0 + CHUNK * k
                                rhs = h1[gi][n][0:9 * s, off:off + CHUNK] \
                                    .rearrange("p (r c) -> p r c",
                                               c=PW)[:, :, 0:W]
                                nc.tensor.matmul(
                                    pt[:],
                                    wt2_sb[gi][:, j * 128:(j + 1) * 128],
                                    rhs, start=(idx == 0),
                                    stop=(idx == nmm - 1))
                                idx += 1
                        dst = h2[j][n][:, 8 * k * W:(8 * k + 8) * W] \
                            .rearrange("p (r c) -> p r c", c=W)
                        src = pt[:, 0:8 * W].rearrange("p (r c) -> p r c", c=W)
                        nc.scalar.activation(
                            dst, src, AF.Copy,
                            accum_out=acc2[(j, "s")][:, col:col + 1])
                        sq = sqp.tile([128, 8 * W], f32, tag="sq", name="sq")
                        c2_last = nc.scalar.activation(
                            sq[:, :].rearrange("p (r c) -> p r c", c=W),
                            dst, AF.Square,
                            accum_out=acc2[(j, "q")][:, col:col + 1])

            # ---- BN2 stats -> exchange -> affine ----
            for j in range(2):
                red2 = emit_stats(acc2[(j, "s")], acc2[(j, "q")], packed2,
                                  2 * j, 128)
            nc.vector.tensor_copy(rv2[:, 0:4], packed2[:])
            tr2 = gp_order(nc.gpsimd.trigger_dma(count=7))
            bass_mod._add_dep_helper(tr2.ins, red2.ins, sync=True,
                                     reason="stats2 ready")

            gl2 = const.tile([128, 4], f32, tag="gl2", name="gl2")
            rec2 = nc.vector.tensor_reduce(
                gl2[:], rv2[:, 0:32].rearrange("p (s c) -> p c s", c=4),
                axis=AX.X, op=ALU.add)
            defer_wait(rec2, rsem[1], 14)
            bass_mod._add_dep_helper(rec2.ins, c2_last.ins, sync=True,
                                     reason="recv after conv phase")
            sc2, bi2 = bn_affine(gl2, 2, SW, SW + 2, "2")

            # ---- tail: out = relu(sc2*h2 + bi2 + x), stream to DRAM.
            # The 32 elementwise ops are spread over DVE/GPSIMD/ACT so no
            # single engine's serial chain gates the (DMA-bound) tail. ----
            for p, (n, rh, j) in enumerate((n, rh, j) for n in range(NL)
                                           for rh in range(2)
                                           for j in range(2)):
                r0 = rh * HALF_ROWS
                xv = interior(x_pad[j][n], (r0 + 1) * PW + 1, HALF_ROWS)
                h2v = h2[j][n][:, r0 * W:r0 * W + HALF_ELEMS].rearrange(
                    "p (r c) -> p r c", c=W)
                tb = otp.tile([128, HALF_ELEMS], bf16, tag="tb",
                              name="tb", bufs=2)
                tbv = tb[:, :].rearrange("p (r c) -> p r c", c=W)
                nc.vector.scalar_tensor_tensor(
                    tbv, h2v, sc2[:, j:j + 1], xv, ALU.mult, ALU.add)
                pool = otp if p % 2 == 0 else xst
                ot = pool.tile([128, HALF_ELEMS], f32,
                               tag="ot" if p % 2 == 0 else "xs",
                               name="ot")
                nc.scalar.activation(ot[:], tb[:], AF.Relu,
                                     bias=bi2[:, j:j + 1], scale=1.0)
                nc.sync.dma_start(
                    out_d[n, j * 128:(j + 1) * 128, r0:r0 + HALF_ROWS, :],
                    ot[:])

    # patch the reserved wait slots to their real thresholds now that
    # scheduling is done (the single-core scheduling simulator cannot
    # satisfy remote increments)
    for bi, sem, val in deferred_waits:
        patched = False
        for wv in bi.ins.sync_info.on_wait:
            if wv.id == sem.num and wv.wait_value == 0:
                wv.wait_value = val
                patched = True
                break
        assert patched, f"deferred wait not found on {bi.ins.name}"

    nc.compile()
    return nc


def kernel(x, W1, W2, gamma1, beta1, gamma2, beta2, mask1, mask2,
           _trace=False, _trace_kwargs=None):
    from concourse.bass_utils import run_bass_kernel_spmd

    mask1 = np.asarray(mask1, np.float32)
    mask2 = np.asarray(mask2, np.float32)
    beta1 = np.asarray(beta1, np.float32)
    groups, kinds = _make_plan(mask1, beta1)
    key = (tuple(len(g) for g in groups), tuple(kinds))
    if _cache.get("key") != key:
        _cache["nc"] = _build(groups, kinds)
        _cache["key"] = key
    nc = _cache["nc"]

    packs = _pack_weights(np.asarray(W1, np.float32),
                          np.asarray(W2, np.float32), mask2, groups, kinds)
    aff = _pack_aff(np.asarray(gamma1, np.float32), beta1,
                    np.asarray(gamma2, np.float32),
                    np.asarray(beta2, np.float32), groups)
    x = np.ascontiguousarray(np.asarray(x, np.float32))

    in_maps = [dict(packs, x=x[i * NL:(i + 1) * NL], aff=aff)
               for i in range(N_CORES)]
    kw = {}
    if _trace:
        kw = dict(trace=True, **(_trace_kwargs or {}))
    res = run_bass_kernel_spmd(nc, in_maps, core_ids=list(range(N_CORES)), **kw)
    out = np.concatenate([res.results[i]["out"] for i in range(N_CORES)],
                         axis=0)
    _cache["last_results"] = res
    return out



# revision 20
# speedup vs baseline: 1.2389x; 1.2389x over previous
"""Trainium2 Bass kernel for a ResNet BasicBlock (dense CNN, sync-BN).

Reference computation (training-mode BN, batch stats over (N,H,W)):
    h = conv3x3(x, W1) * mask1            # structured channel pruning
    h = relu(bn(h, gamma1, beta1))
    h = conv3x3(h, W2) * mask2
    h = bn(h, gamma2, beta2)
    out = relu(h + x)                      # identity shortcut

Shapes: x [32, 256, 56, 56] f32, W [256, 256, 3, 3] f32.

Strategy: data-parallel over batch N across 8 NeuronCores (4 images per
core), weights replicated.  BN batch statistics are synchronized with a
single 8-destination remote-DMA broadcast per exchange: every core
lands its packed stats tile in slot `partition_id` of each peer's
receive tile (the slot offset is a runtime register), so the exchange
is one descriptor batch on all 16 DMA engines instead of 7 serialized
single-destination sends.  Slot 0 of the destination list is a dummy
(no self-loopback); each core's own contribution is added locally when
the global stats are assembled, and receive tiles are pre-zeroed so
the unwritten own-slot sums as zero.

Channel sparsity: mask1 zeroes ~half of conv1's output channels, and
(when beta1 <= 0 on those channels) the corresponding h1 channels are
exactly zero, so conv1 only computes the a1=|mask1| active channels and
conv2 only consumes them.  Active channels are packed into partition
groups of <=128 via a host-side permutation folded into the weights:
  - conv1 output groups: [128, a1-128]; the small overflow group's
    conv2 contribution is evaluated as ONE im2col matmul per chunk
    (K = 9 taps * (a1-128) channels <= 128) against a plane holding 9
    tap-shifted replicas of the overflow channels (built by SBUF->SBUF
    DMAs; the center tap block sits at partition 0 so compute APs stay
    partition-aligned, and the block order is folded into the weights).
  - conv2 output stays in TRUE channel layout (inactive mask2 rows are
    zeroed in the weights), so the residual tail needs no scatter.

conv2 runs output-half-major (j=0 fully, then j=1): half 0's BN2 stats
exchange and its entire tail (affine + relu + residual + DRAM store,
computed on the otherwise-idle gpsimd engine) overlap half 1's matmul
stream, so only half 1's tail remains after the last matmul.

Per-core layout: zero-padded 58x58 bf16 planes so each conv tap is an
offset shift; 7 chunks of 8 output rows per image so a chunk fits a
PSUM bank.  Head loads / BN1 applies are emitted interleaved with the
conv work (one image of lookahead) so the strict-FIFO ACT/DVE queues
never dam up behind bulk phases.
"""

import numpy as np
import ml_dtypes

# ---- problem constants (hardcoded; kernel.py must be self-contained) ----
N_TOT, C, H, W = 32, 256, 56, 56
N_CORES = 8
NL = N_TOT // N_CORES          # images per core
PW = H + 2                     # padded row stride (58)
PLANE = PW * PW + 4            # padded plane floats + 4 spare for tap overreads
STRIP0 = PW + 1                # first interior output position (59)
CHUNK = 8 * PW                 # 464: 8 output rows per chunk
NCHUNK = 7                     # 7 chunks * 8 rows = 56 rows
HW = H * W                     # 3136
HALF_ROWS = 28                 # row granularity for x/out streaming DMAs
HALF_ELEMS = HALF_ROWS * W     # 1568
COUNT = N_TOT * HW             # sync-BN element count per channel
CL = NL * HW                   # local per-core element count per channel
EPS = 1e-5

_BF16 = ml_dtypes.bfloat16

_cache = {}

TAPS = [(ky, kx) for ky in range(3) for kx in range(3)]
BORD = [4, 0, 1, 2, 3, 5, 6, 7, 8]   # i2c plane tap-block order, center first


def _make_plan(mask1, beta1):
    """Conv1 output channel groups (== conv2 input groups), true-channel ids."""
    act1 = np.flatnonzero(mask1 != 0)
    inact = np.flatnonzero(mask1 == 0)
    if np.any(np.maximum(beta1[inact], 0.0) != 0.0) or len(act1) == 0:
        act1 = np.arange(C)    # dense fallback: every channel treated live
    groups = [act1[i:i + 128] for i in range(0, len(act1), 128)]
    kinds = ["i2c" if (9 * len(g) <= 128 and len(g) < 128) else "full"
             for g in groups]
    return groups, kinds


def _pack_weights(W1, W2, mask2, groups, kinds):
    m2 = mask2.reshape(2, 128).astype(np.float32)
    packs = {}
    for gi, (g, kind) in enumerate(zip(groups, kinds)):
        s = len(g)
        blocks = []
        for h in range(2):
            for (ky, kx) in TAPS:
                blocks.append(W1[g, 128 * h:128 * h + 128, ky, kx].T)  # [ci,co]
        packs[f"wt1_{gi}"] = np.ascontiguousarray(
            np.concatenate(blocks, axis=1)).astype(_BF16)      # [128, 18*s]
        if kind == "full":
            blocks = []
            for j in range(2):
                for (ky, kx) in TAPS:
                    blk = (W2[128 * j:128 * j + 128, :, ky, kx][:, g]
                           * m2[j][:, None]).T                  # [s, 128co]
                    blocks.append(blk)
            packs[f"wt2m_{gi}"] = np.ascontiguousarray(
                np.concatenate(blocks, axis=1)).astype(_BF16)   # [s, 18*128]
        else:
            # block order: center tap first so the plane's compute-written
            # rows sit at partition base 0 (engine AP alignment rule)
            arr = np.zeros((9 * s, 256), np.float32)
            for b, t in enumerate(BORD):
                ky, kx = TAPS[t]
                for j in range(2):
                    arr[b * s:(b + 1) * s, j * 128:(j + 1) * 128] = \
                        (W2[128 * j:128 * j + 128, :, ky, kx][:, g]
                         * m2[j][:, None]).T
            packs[f"wt2o_{gi}"] = arr.astype(_BF16)             # [9s, 256]
    return packs


def _pack_aff(gamma1, beta1, gamma2, beta2, groups):
    G = len(groups)
    out = np.zeros((128, 2 * G + 4), np.float32)
    for gi, g in enumerate(groups):
        s = len(g)
        reps = 9 if (9 * s <= 128 and s < 128) else 1
        out[:s * reps, gi] = np.tile(gamma1[g], reps)
        out[:s * reps, G + gi] = np.tile(beta1[g], reps)
    g2 = np.asarray(gamma2, np.float32).reshape(2, 128)
    b2 = np.asarray(beta2, np.float32).reshape(2, 128)
    out[:, 2 * G + 0] = g2[0]
    out[:, 2 * G + 1] = g2[1]
    out[:, 2 * G + 2] = b2[0]
    out[:, 2 * G + 3] = b2[1]
    return out


def _build(groups, kinds):
    import concourse.bass as bass_mod
    import concourse.bacc as bacc
    import concourse.mybir as mybir
    import concourse.tile as tile

    f32 = mybir.dt.float32
    bf16 = mybir.dt.bfloat16
    AX = mybir.AxisListType
    ALU = mybir.AluOpType
    AF = mybir.ActivationFunctionType

    G = len(groups)
    sizes = [len(g) for g in groups]
    # stats-exchange column layouts.  "full" groups carry (sum, sumsq) column
    # pairs; a trailing i2c group packs its s sums at partitions [0:s] and its
    # s sumsqs at partitions [s:2s] of ONE column (2s <= 128).
    n_i2c = sum(1 for k in kinds if k == "i2c")
    SW1 = 2 * (G - n_i2c) + n_i2c        # BN1 exchange width (3 for [128,9])

    nc = bacc.Bacc("TRN2", target_bir_lowering=False, debug=False,
                   num_devices=N_CORES)

    x_d = nc.dram_tensor("x", [NL, C, H, W], f32, kind="ExternalInput")
    wt1_d = [nc.dram_tensor(f"wt1_{gi}", [128, 18 * sizes[gi]], bf16,
                            kind="ExternalInput") for gi in range(G)]
    wt2_d = []
    for gi in range(G):
        if kinds[gi] == "full":
            wt2_d.append(nc.dram_tensor(f"wt2m_{gi}", [sizes[gi], 18 * 128],
                                        bf16, kind="ExternalInput"))
        else:
            wt2_d.append(nc.dram_tensor(f"wt2o_{gi}", [9 * sizes[gi], 256],
                                        bf16, kind="ExternalInput"))
    aff_d = nc.dram_tensor("aff", [128, 2 * G + 4], f32, kind="ExternalInput")
    out_d = nc.dram_tensor("out", [NL, C, H, W], f32, kind="ExternalOutput")

    replica_groups = [list(range(N_CORES))]

    def interior(tile_ap, base, nrows):
        """[p, nrows, 56] strided view (row stride PW) starting at `base`."""
        v = tile_ap[:, base:base + nrows * PW].rearrange(
            "p (r c) -> p r c", c=PW)
        return v[:, :, 0:W]

    with tile.TileContext(nc) as tc:
        import contextlib
        with contextlib.ExitStack() as ctx:
            const = ctx.enter_context(tc.tile_pool(name="const", bufs=1))
            psum = ctx.enter_context(tc.tile_pool(name="psum", bufs=6,
                                                  space="PSUM"))
            psumb = ctx.enter_context(tc.tile_pool(name="psumb", bufs=2,
                                                   space="PSUM"))
            xst = ctx.enter_context(tc.tile_pool(name="xst", bufs=3))
            otp = ctx.enter_context(tc.tile_pool(name="otp", bufs=2))
            sqp = ctx.enter_context(tc.tile_pool(name="sqp", bufs=2))

            wt1_sb = [const.tile([128, 18 * sizes[gi]], bf16,
                                 tag=f"wt1_{gi}", name=f"wt1_{gi}")
                      for gi in range(G)]
            wt2_sb = []
            for gi in range(G):
                if kinds[gi] == "full":
                    wt2_sb.append(const.tile([sizes[gi], 18 * 128], bf16,
                                             tag=f"wt2_{gi}", name=f"wt2_{gi}"))
                else:
                    wt2_sb.append(const.tile([9 * sizes[gi], 256], bf16,
                                             tag=f"wt2_{gi}", name=f"wt2_{gi}"))
            for gi in range(G):
                nc.sync.dma_start(wt1_sb[gi][:], wt1_d[gi][:])
                nc.sync.dma_start(wt2_sb[gi][:], wt2_d[gi][:])
            aff_sb = const.tile([128, 2 * G + 4], f32, tag="aff", name="aff")
            nc.sync.dma_start(aff_sb[:], aff_d[:])

            # persistent per-image planes
            x_pad = [[const.tile([128, PLANE], bf16, tag=f"xp{j}_{n}",
                                 name=f"xp{j}_{n}")
                      for n in range(NL)] for j in range(2)]
            h1 = []                       # per group: list over images
            for gi in range(G):
                rows = 9 * sizes[gi] if kinds[gi] == "i2c" else sizes[gi]
                h1.append([const.tile([rows, PLANE], bf16, tag=f"h1{gi}_{n}",
                                      name=f"h1{gi}_{n}")
                           for n in range(NL)])
            h2 = [[const.tile([128, HW], bf16, tag=f"h2{j}_{n}",
                              name=f"h2{j}_{n}")
                   for n in range(NL)] for j in range(2)]

            # zero the non-interior positions of padded planes (i2c planes:
            # center block only; other blocks are fully DMA-overwritten with
            # shifted copies whose source pads are these zeros)
            # pad zeroing on gpsimd (idle at entry) so the DVE FIFO is
            # free for the x casts and the first matmul can issue early
            def zero_pads(t, s):
                tt = t[0:s]
                nc.gpsimd.memset(tt[:, 0:STRIP0], 0.0)
                pairs = tt[:, 2 * PW - 1:2 * PW - 1 + 56 * PW].rearrange(
                    "p (r c) -> p r c", c=PW)[:, :, 0:2]
                nc.gpsimd.memset(pairs, 0.0)
                nc.gpsimd.memset(tt[:, STRIP0 + 56 * PW:PLANE], 0.0)

            # receive tiles for the stats exchanges: pre-zeroed (before any
            # peer can arrive -- peers only send after their conv1, >200us
            # out) so the unwritten own slot contributes zero to the sum.
            rv1 = const.tile([128, 8 * SW1], f32, tag="rv1", name="rv1")
            rv2 = [const.tile([128, 16], f32, tag=f"rv2{j}", name=f"rv2{j}")
                   for j in range(2)]
            nc.gpsimd.memset(rv1[:], 0.0)
            nc.gpsimd.memset(rv2[0][:], 0.0)
            nc.gpsimd.memset(rv2[1][:], 0.0)

            for j in range(2):
                for n in range(NL):
                    zero_pads(x_pad[j][n], 128)
            for gi in range(G):
                for n in range(NL):
                    zero_pads(h1[gi][n], sizes[gi])

            # ---- cross-core stats exchange plumbing (SBUF remote DMA) ----
            # Three exchanges (BN1, BN2 half0, BN2 half1).  Each is ONE
            # 8-destination remote broadcast: core `p` lands its packed
            # [128, w] stats tile in slot p of every core's receive tile
            # (the slot offset is p*w, computed at descgen time from the
            # partition-id register).  Each arrival bumps rsem by +2, so a
            # full exchange is rsem >= 14 (7 peers; own slot stays zero).
            # Descriptors are PRE-GENERATED here; a single gpsimd
            # trigger_dma fires each when its stats tile is ready.
            rsem = [nc.alloc_semaphore(f"rst{i}") for i in range(3)]
            lsem = nc.alloc_semaphore("lst")
            _gp_prev = [None]
            deferred_waits = []

            def gp_order(bi):
                if _gp_prev[0] is not None:
                    bass_mod._add_dep_helper(bi.ins, _gp_prev[0].ins,
                                             sync=False,
                                             reason="stats-exchange order")
                _gp_prev[0] = bi
                return bi

            nc._bir_kernel_barrier_sem_replica_groups.extend(
                set(g) for g in replica_groups)

            def defer_wait(bi, sem, val):
                bi._wait_ge(sem, 0)
                deferred_waits.append((bi, sem, val))
                return bi

            # sems persist across NEFF executions: clear them as soon as all
            # cores have entered (peers send >100us later, after conv1)
            for i, s in enumerate(rsem + [lsem]):
                cl = gp_order(nc.gpsimd.sem_clear(s))
                if i == 0:
                    defer_wait(cl, nc._bir_kernel_barrier_sem,
                               nc.bir_kernel_barrier_sem_inc)

            pid = nc.gpsimd.partition_id()

            # stats tiles written at conv end; descriptors reference them now.
            # rdests slot 0 is a dummy: a core never sends to itself (XOR
            # distance 0); its own stats are added locally at unpack time.
            packed1 = const.tile([128, SW1], f32, tag="pk1", name="pk1")
            packed2 = [const.tile([128, 2], f32, tag=f"pk2{j}", name=f"pk2{j}")
                       for j in range(2)]
            peer_dests = [None] + [(0, d) for d in range(1, 8)]
            for ex, (pk, rv, w) in enumerate(((packed1, rv1, SW1),
                                              (packed2[0], rv2[0], 2),
                                              (packed2[1], rv2[1], 2))):
                dst = rv[:, 0:w]
                dst.offset = pid * w + dst.offset
                gp_order(nc.gpsimd.remote_dma_broadcast(
                    dst, pk[:], remote_sem=rsem[ex], local_sem=lsem,
                    rdests=peer_dests))

            # per-(image,chunk) (sum, sumsq) accumulator columns, filled by
            # the ACT drain copies' accum_out and a paired ACT square op
            acc1 = {(gi, sq): const.tile([sizes[gi], NL * NCHUNK], f32,
                                         tag=f"a1{gi}{sq}", name=f"a1{gi}{sq}")
                    for gi in range(G) for sq in ("s", "q")}
            acc2 = {(j, sq): const.tile([128, NL * NCHUNK], f32,
                                        tag=f"a2{j}{sq}", name=f"a2{j}{sq}")
                    for j in range(2) for sq in ("s", "q")}

            # ---- head: stream x in (all DMAs up front; staging ring 4),
            # casts emitted per image inside the conv1 loop below ----
            head_xs = []
            for n in range(NL):
                for rh in range(2):
                    for j in range(2):
                        r0 = rh * HALF_ROWS
                        xs = xst.tile([128, HALF_ELEMS], f32, tag="xs",
                                      name="xs")
                        nc.sync.dma_start(
                            xs[:],
                            x_d[n, j * 128:(j + 1) * 128, r0:r0 + HALF_ROWS, :])
                        head_xs.append((n, rh, j, xs))

            def emit_casts(n):
                for (nn, rh, j, xs) in head_xs:
                    if nn != n:
                        continue
                    r0 = rh * HALF_ROWS
                    dst = interior(x_pad[j][nn], (r0 + 1) * PW + 1, HALF_ROWS)
                    src = xs[:, :].rearrange("p (r c) -> p r c", c=W)
                    nc.vector.tensor_copy(dst, src)

            # ---- conv1: per chunk, one 18-matmul run per output group
            # (runs kept contiguous per PSUM group so the PE's LDWEIGHTS
            # pull-ahead pipelining stays active) ----
            emit_casts(0)
            emit_casts(1)
            c1_last = None
            for n in range(NL):
                if n + 2 < NL:
                    emit_casts(n + 2)
                for k in range(NCHUNK):
                    col = n * NCHUNK + k
                    for gi in range(G):
                        s = sizes[gi]
                        pool = psum if kinds[gi] == "full" else psumb
                        tag = "ps" if kinds[gi] == "full" else "psb"
                        pt = pool.tile([s, 8 * W], f32, tag=tag, name=tag)
                        for idx, (hh, (ky, kx)) in enumerate(
                                (hh, t) for hh in range(2) for t in TAPS):
                            dq = (ky - 1) * PW + (kx - 1)
                            off = STRIP0 + CHUNK * k + dq
                            rhs = x_pad[hh][n][:, off:off + CHUNK].rearrange(
                                "p (r c) -> p r c", c=PW)[:, :, 0:W]
                            nc.tensor.matmul(
                                pt[:], wt1_sb[gi][:, idx * s:(idx + 1) * s],
                                rhs, start=(idx == 0), stop=(idx == 17))
                        base = (1 + 8 * k) * PW + 1
                        dst = interior(h1[gi][n][0:s], base, 8)
                        src = pt[:, 0:8 * W].rearrange("p (r c) -> p r c", c=W)
                        nc.scalar.activation(
                            dst, src, AF.Copy,
                            accum_out=acc1[(gi, "s")][:, col:col + 1])
                        sq = sqp.tile([128, 8 * W], f32, tag="sq", name="sq")
                        c1_last = nc.scalar.activation(
                            sq[0:s, :].rearrange("p (r c) -> p r c", c=W),
                            dst, AF.Square,
                            accum_out=acc1[(gi, "q")][:, col:col + 1])

            # ---- BN1 stats: aggregate into the packed wire layout and fire
            # the broadcast IMMEDIATELY (before the bulk replication DMAs
            # below hit the queues) ----
            # wire layout per group: "full" -> (sum, sumsq) column pair;
            # "i2c" -> one column with sums at partitions [0:s] and sumsqs
            # at partitions [32:32+s] (engine APs must start at a partition
            # base in {0,32,64,96}, so s <= 32 rides in one column).
            stats_ready = None
            colp = 0
            for gi in range(G):
                s = sizes[gi]
                if kinds[gi] == "full":
                    nc.vector.tensor_reduce(
                        packed1[0:s, colp:colp + 1], acc1[(gi, "s")][:],
                        axis=AX.X, op=ALU.add)
                    stats_ready = nc.vector.tensor_reduce(
                        packed1[0:s, colp + 1:colp + 2], acc1[(gi, "q")][:],
                        axis=AX.X, op=ALU.add)
                    colp += 2
                else:
                    nc.vector.tensor_reduce(
                        packed1[0:s, colp:colp + 1], acc1[(gi, "s")][:],
                        axis=AX.X, op=ALU.add)
                    stats_ready = nc.vector.tensor_reduce(
                        packed1[32:32 + s, colp:colp + 1], acc1[(gi, "q")][:],
                        axis=AX.X, op=ALU.add)
                    colp += 1
            tr1 = gp_order(nc.gpsimd.trigger_dma(count=1))
            bass_mod._add_dep_helper(tr1.ins, stats_ready.ins, sync=True,
                                     reason="stats1 ready")

            # ---- wait for all 8 contributions, sum slots -> global stats ----
            gl1 = const.tile([128, SW1], f32, tag="gl1", name="gl1")
            rec1 = nc.vector.tensor_reduce(
                gl1[:], rv1[:, 0:8 * SW1].rearrange("p (s c) -> p c s", c=SW1),
                axis=AX.X, op=ALU.add)
            defer_wait(rec1, rsem[0], 14)
            bass_mod._add_dep_helper(rec1.ins, c1_last.ins, sync=True,
                                     reason="recv after conv phase")

            # unpack to the affine layout gl1x [128, 2G]: (sum, sumsq) column
            # pairs per group, adding this core's own packed stats (its rv
            # slot is the zeroed dummy).  i2c stats stay on the base rows
            # [0:s]; the BN1 apply touches only the center tap block and the
            # tap replicas are copied AFTER the apply (see conv2 loop).
            gl1x = const.tile([128, 2 * G], f32, tag="gl1x", name="gl1x")
            colp = 0
            gx = 0
            for gi in range(G):
                s = sizes[gi]
                if kinds[gi] == "full":
                    nc.vector.tensor_tensor(gl1x[0:s, gx:gx + 2],
                                            gl1[0:s, colp:colp + 2],
                                            packed1[0:s, colp:colp + 2],
                                            ALU.add)
                    colp += 2
                else:
                    nc.vector.tensor_tensor(
                        gl1x[0:s, gx:gx + 1],
                        gl1[0:s, colp:colp + 1],
                        packed1[0:s, colp:colp + 1], ALU.add)
                    nc.vector.tensor_tensor(
                        gl1x[0:s, gx + 1:gx + 2],
                        gl1[32:32 + s, colp:colp + 1],
                        packed1[32:32 + s, colp:colp + 1], ALU.add)
                    colp += 1
                gx += 2

            # ---- BN affine from global stats (DVE-only; fast rsqrt) ----
            def bn_affine(gl, w, g_col, b_col, sfx):
                """gl [128, 2w] = (sum, sumsq) pairs -> scale/bias [128, w]."""
                glv = gl[:, 0:2 * w].rearrange("p (g c) -> p g c", c=2)
                mean = const.tile([128, w], f32, tag=f"mean{sfx}",
                                  name=f"mean{sfx}")
                nc.vector.tensor_scalar_mul(mean[:], glv[:, :, 0],
                                            1.0 / COUNT)
                var = const.tile([128, w], f32, tag=f"var{sfx}",
                                 name=f"var{sfx}")
                nc.vector.tensor_tensor(var[:], mean[:], mean[:], ALU.mult)
                nc.vector.scalar_tensor_tensor(
                    var[:], glv[:, :, 1], 1.0 / COUNT, var[:],
                    ALU.mult, ALU.subtract)
                nc.vector.tensor_scalar_add(var[:], var[:], EPS)
                y = const.tile([128, w], f32, tag=f"y{sfx}", name=f"y{sfx}")
                nc.scalar.activation(y[:], var[:], AF.Sqrt)
                nc.vector.reciprocal(y[:], y[:])
                sc = const.tile([128, w], f32, tag=f"sc{sfx}", name=f"sc{sfx}")
                nc.vector.tensor_tensor(sc[:], aff_sb[:, g_col:g_col + w],
                                        y[:], ALU.mult)
                bi = const.tile([128, w], f32, tag=f"bi{sfx}", name=f"bi{sfx}")
                nc.vector.tensor_tensor(bi[:], mean[:], sc[:], ALU.mult)
                nc.vector.tensor_tensor(bi[:], aff_sb[:, b_col:b_col + w],
                                        bi[:], ALU.subtract)
                return sc, bi

            sc1, bi1 = bn_affine(gl1x, G, 0, G, "1")

            # ---- per image: BN1 apply (ACT relu, rh-major so conv2's first
            # chunks unblock quickly).  i2c groups apply only the center tap
            # block; the 8 tap-shifted replicas are DMA-copied from the
            # POST-relu center block right after (overlapping conv2's
            # full-group matmuls, which don't consume them). ----
            def emit_apply(n):
                for rh in range(2):
                    base = (1 + rh * HALF_ROWS) * PW + 1
                    for gi in range(G):
                        s = sizes[gi]
                        v = interior(h1[gi][n][0:s], base, HALF_ROWS)
                        nc.scalar.activation(
                            v, v, AF.Relu,
                            bias=bi1[0:s, gi:gi + 1],
                            scale=sc1[0:s, gi:gi + 1])

            def emit_repl(n):
                for gi in range(G):
                    if kinds[gi] != "i2c":
                        continue
                    s = sizes[gi]
                    for b, t in enumerate(BORD):
                        if b == 0:
                            continue
                        ky, kx = TAPS[t]
                        dq = (ky - 1) * PW + (kx - 1)
                        nc.sync.dma_start(
                            h1[gi][n][b * s:(b + 1) * s,
                                      STRIP0:STRIP0 + 56 * PW],
                            h1[gi][n][0:s,
                                      STRIP0 + dq:STRIP0 + 56 * PW + dq])

            # ---- conv2, output-half-major.  Half j's stats fire as soon as
            # its last chunk drains; half 0's exchange and tail overlap half
            # 1's matmuls (tail compute on gpsimd, which is idle here). ----
            sc2 = [None, None]
            bi2 = [None, None]

            def emit_conv2_chunk(n, k, j):
                col = n * NCHUNK + k
                pt = psum.tile([128, 8 * W], f32, tag="ps", name="ps")
                nmm = sum(9 if kinds[gi] == "full" else 1
                          for gi in range(G))
                idx = 0
                for gi in range(G):
                    s = sizes[gi]
                    if kinds[gi] == "full":
                        for t, (ky, kx) in enumerate(TAPS):
                            dq = (ky - 1) * PW + (kx - 1)
                            off = STRIP0 + CHUNK * k + dq
                            rhs = h1[gi][n][0:s, off:off + CHUNK] \
                                .rearrange("p (r c) -> p r c",
                                           c=PW)[:, :, 0:W]
                            nc.tensor.matmul(
                                pt[:],
                                wt2_sb[gi][:, (j * 9 + t) * 128:
                                           (j * 9 + t + 1) * 128],
                                rhs, start=(idx == 0),
                                stop=(idx == nmm - 1))
                            idx += 1
                    else:
                        off = STRIP0 + CHUNK * k
                        rhs = h1[gi][n][0:9 * s, off:off + CHUNK] \
                            .rearrange("p (r c) -> p r c",
                                       c=PW)[:, :, 0:W]
                        nc.tensor.matmul(
                            pt[:],
                            wt2_sb[gi][:, j * 128:(j + 1) * 128],
                            rhs, start=(idx == 0),
                            stop=(idx == nmm - 1))
                        idx += 1
                dst = h2[j][n][:, 8 * k * W:(8 * k + 8) * W] \
                    .rearrange("p (r c) -> p r c", c=W)
                src = pt[:, 0:8 * W].rearrange("p (r c) -> p r c", c=W)
                nc.scalar.activation(
                    dst, src, AF.Copy,
                    accum_out=acc2[(j, "s")][:, col:col + 1])
                sq = sqp.tile([128, 8 * W], f32, tag="sq", name="sq")
                return nc.scalar.activation(
                    sq[:, :].rearrange("p (r c) -> p r c", c=W),
                    dst, AF.Square,
                    accum_out=acc2[(j, "q")][:, col:col + 1])

            def emit_stats2(j, c2_last):
                nc.vector.tensor_reduce(
                    packed2[j][:, 0:1], acc2[(j, "s")][:], axis=AX.X,
                    op=ALU.add)
                red = nc.vector.tensor_reduce(
                    packed2[j][:, 1:2], acc2[(j, "q")][:], axis=AX.X,
                    op=ALU.add)
                tr = gp_order(nc.gpsimd.trigger_dma(count=1))
                bass_mod._add_dep_helper(tr.ins, red.ins, sync=True,
                                         reason=f"stats2{j} ready")
                gl = const.tile([128, 2], f32, tag=f"gl2{j}", name=f"gl2{j}")
                rec = nc.vector.tensor_reduce(
                    gl[:], rv2[j][:, 0:16].rearrange("p (s c) -> p c s", c=2),
                    axis=AX.X, op=ALU.add)
                defer_wait(rec, rsem[1 + j], 14)
                bass_mod._add_dep_helper(rec.ins, c2_last.ins, sync=True,
                                         reason="recv after conv phase")
                nc.vector.tensor_tensor(gl[:], gl[:], packed2[j][:], ALU.add)
                sc2[j], bi2[j] = bn_affine(gl, 1, 2 * G + j, 2 * G + 2 + j,
                                           f"2{j}")

            def emit_tail_piece(n, rh, j, eng):
                """out[n, half j, row-half rh] = relu(sc2*h2 + bi2 + x).

                eng: 'dv2' = both elementwise ops on DVE (used for half 0,
                which overlaps half 1's matmul stream where ACT carries the
                drain traffic and DVE is idle); 'dv' = DVE stt + ACT relu
                (the final tail, where both engines are free).
                """
                r0 = rh * HALF_ROWS
                xv = interior(x_pad[j][n], (r0 + 1) * PW + 1, HALF_ROWS)
                h2v = h2[j][n][:, r0 * W:r0 * W + HALF_ELEMS].rearrange(
                    "p (r c) -> p r c", c=W)
                tb = otp.tile([128, HALF_ELEMS], bf16, tag="tb",
                              name="tb", bufs=2)
                tbv = tb[:, :].rearrange("p (r c) -> p r c", c=W)
                pool = otp if (rh + j) % 2 == 0 else xst
                ot = pool.tile([128, HALF_ELEMS], f32,
                               tag="ot" if (rh + j) % 2 == 0 else "xs",
                               name="ot")
                nc.vector.scalar_tensor_tensor(
                    tbv, h2v, sc2[j][:, 0:1], xv, ALU.mult, ALU.add)
                if eng == "dv2":
                    nc.vector.tensor_scalar(
                        ot[:], tb[:], bi2[j][:, 0:1], 0.0,
                        ALU.add, ALU.max)
                else:
                    nc.scalar.activation(ot[:], tb[:], AF.Relu,
                                         bias=bi2[j][:, 0:1], scale=1.0)
                nc.sync.dma_start(
                    out_d[n, j * 128:(j + 1) * 128, r0:r0 + HALF_ROWS, :],
                    ot[:])

            # half 0: BN1 applies + tap replication pipeline one image ahead
            # of the matmuls
            emit_apply(0)
            emit_repl(0)
            c2_last = None
            for n in range(NL):
                if n + 1 < NL:
                    emit_apply(n + 1)
                    emit_repl(n + 1)
                for k in range(NCHUNK):
                    c2_last = emit_conv2_chunk(n, k, 0)
            emit_stats2(0, c2_last)

            # half 1: interleave half 0's tail (gpsimd + DMA, both idle
            # during the matmul stream).  One image of lag keeps the
            # strict-FIFO queues from damming behind the exchange wait.
            tail0 = [(n, rh) for n in range(NL) for rh in range(2)]
            ti = 0
            for n in range(NL):
                for k in range(NCHUNK):
                    c2_last = emit_conv2_chunk(n, k, 1)
                while ti < len(tail0) and tail0[ti][0] <= n - 2:
                    tn, trh = tail0[ti]
                    emit_tail_piece(tn, trh, 0, "dv2")
                    ti += 1
            emit_stats2(1, c2_last)
            while ti < len(tail0):
                tn, trh = tail0[ti]
                emit_tail_piece(tn, trh, 0, "dv2")
                ti += 1

            # half 1's tail: split across DVE/ACT and gpsimd
            for p, (n, rh) in enumerate((n, rh) for n in range(NL)
                                        for rh in range(2)):
                emit_tail_piece(n, rh, 1, "dv2" if p % 2 == 0 else "dv")

    # patch the reserved wait slots to their real thresholds now that
    # scheduling is done (the single-core scheduling simulator cannot
    # satisfy remote increments)
    for bi, sem, val in deferred_waits:
        patched = False
        for wv in bi.ins.sync_info.on_wait:
            if wv.id == sem.num and wv.wait_value == 0:
                wv.wait_value = val
                patched = True
                break
        assert patched, f"deferred wait not found on {bi.ins.name}"

    nc.compile()
    return nc


def kernel(x, W1, W2, gamma1, beta1, gamma2, beta2, mask1, mask2,
           _trace=False, _trace_kwargs=None):
    from concourse.bass_utils import run_bass_kernel_spmd

    mask1 = np.asarray(mask1, np.float32)
    mask2 = np.asarray(mask2, np.float32)
    beta1 = np.asarray(beta1, np.float32)
    groups, kinds = _make_plan(mask1, beta1)
    key = (tuple(len(g) for g in groups), tuple(kinds))
    if _cache.get("key") != key:
        _cache["nc"] = _build(groups, kinds)
        _cache["key"] = key
    nc = _cache["nc"]

    packs = _pack_weights(np.asarray(W1, np.float32),
                          np.asarray(W2, np.float32), mask2, groups, kinds)
    aff = _pack_aff(np.asarray(gamma1, np.float32), beta1,
                    np.asarray(gamma2, np.float32),
                    np.asarray(beta2, np.float32), groups)
    x = np.ascontiguousarray(np.asarray(x, np.float32))

    in_maps = [dict(packs, x=x[i * NL:(i + 1) * NL], aff=aff)
               for i in range(N_CORES)]
    kw = {}
    if _trace:
        kw = dict(trace=True, **(_trace_kwargs or {}))
    res = run_bass_kernel_spmd(nc, in_maps, core_ids=list(range(N_CORES)), **kw)
    out = np.concatenate([res.results[i]["out"] for i in range(N_CORES)],
                         axis=0)
    _cache["last_results"] = res
    return out


# revision 29
# speedup vs baseline: 1.5874x; 1.2813x over previous
"""Trainium2 Bass kernel for a ResNet BasicBlock (dense CNN, sync-BN).

Reference computation (training-mode BN, batch stats over (N,H,W)):
    h = conv3x3(x, W1) * mask1            # structured channel pruning
    h = relu(bn(h, gamma1, beta1))
    h = conv3x3(h, W2) * mask2
    h = bn(h, gamma2, beta2)
    out = relu(h + x)                      # identity shortcut

Shapes: x [32, 256, 56, 56] f32, W [256, 256, 3, 3] f32.

Strategy: data-parallel over batch N across 8 NeuronCores (4 images per
core), weights replicated.  BN batch statistics are synchronized with a
single 8-destination remote-DMA broadcast per exchange: every core
lands its packed stats tile in slot `partition_id` of each peer's
receive tile (the slot offset is a runtime register), so the exchange
is one descriptor batch on all 16 DMA engines instead of 7 serialized
single-destination sends.  Slot 0 of the destination list is a dummy
(no self-loopback); each core's own contribution is added locally when
the global stats are assembled, and receive tiles are pre-zeroed so
the unwritten own-slot sums as zero.

Channel sparsity: mask1 zeroes ~half of conv1's output channels, and
(when beta1 <= 0 on those channels) the corresponding h1 channels are
exactly zero, so conv1 only computes the a1=|mask1| active channels and
conv2 only consumes them.  Active channels are packed into partition
groups of <=128 via a host-side permutation folded into the weights:
  - conv1 output groups: [128, a1-128]; the small overflow group's
    conv2 contribution is evaluated as ONE im2col matmul per chunk
    (K = 9 taps * (a1-128) channels <= 128) against a plane holding 9
    tap-shifted replicas of the overflow channels (built by SBUF->SBUF
    DMAs; the center tap block sits at partition 0 so compute APs stay
    partition-aligned, and the block order is folded into the weights).
  - conv2 output stays in TRUE channel layout (inactive mask2 rows are
    zeroed in the weights), so the residual tail needs no scatter.

conv2 runs output-half-major (j=0 fully, then j=1): half 0's BN2 stats
exchange and its entire tail (affine + relu + residual + DRAM store,
computed on the otherwise-idle gpsimd engine) overlap half 1's matmul
stream, so only half 1's tail remains after the last matmul.

Per-core layout: zero-padded 58x58 bf16 planes so each conv tap is an
offset shift; 7 chunks of 8 output rows per image so a chunk fits a
PSUM bank.  Head loads / BN1 applies are emitted interleaved with the
conv work (one image of lookahead) so the strict-FIFO ACT/DVE queues
never dam up behind bulk phases.
"""

import numpy as np
import ml_dtypes

# ---- problem constants (hardcoded; kernel.py must be self-contained) ----
N_TOT, C, H, W = 32, 256, 56, 56
N_CORES = 8
NL = N_TOT // N_CORES          # images per core
PW = H + 2                     # padded row stride (58)
PLANE = PW * PW + 4            # padded plane floats + 4 spare for tap overreads
STRIP0 = PW + 1                # first interior output position (59)
CHUNK = 8 * PW                 # 464: 8 output rows per chunk
NCHUNK = 7                     # 7 chunks * 8 rows = 56 rows
HW = H * W                     # 3136
HALF_ROWS = 28                 # row granularity for x/out streaming DMAs
HALF_ELEMS = HALF_ROWS * W     # 1568
COUNT = N_TOT * HW             # sync-BN element count per channel
CL = NL * HW                   # local per-core element count per channel
EPS = 1e-5

_BF16 = ml_dtypes.bfloat16

_cache = {}

TAPS = [(ky, kx) for ky in range(3) for kx in range(3)]
BORD = [4, 0, 1, 2, 3, 5, 6, 7, 8]   # i2c plane tap-block order, center first


def _make_plan(mask1, beta1):
    """Conv1 output channel groups (== conv2 input groups), true-channel ids."""
    act1 = np.flatnonzero(mask1 != 0)
    inact = np.flatnonzero(mask1 == 0)
    if np.any(np.maximum(beta1[inact], 0.0) != 0.0) or len(act1) == 0:
        act1 = np.arange(C)    # dense fallback: every channel treated live
    groups = [act1[i:i + 128] for i in range(0, len(act1), 128)]
    kinds = ["i2c" if (9 * len(g) <= 128 and len(g) < 128) else "full"
             for g in groups]
    return groups, kinds


def _pack_weights(W1, W2, mask2, groups, kinds):
    m2 = mask2.reshape(2, 128).astype(np.float32)
    packs = {}
    for gi, (g, kind) in enumerate(zip(groups, kinds)):
        s = len(g)
        blocks = []
        for h in range(2):
            for (ky, kx) in TAPS:
                blocks.append(W1[g, 128 * h:128 * h + 128, ky, kx].T)  # [ci,co]
        packs[f"wt1_{gi}"] = np.ascontiguousarray(
            np.concatenate(blocks, axis=1)).astype(_BF16)      # [128, 18*s]
        if kind == "full":
            blocks = []
            for j in range(2):
                for (ky, kx) in TAPS:
                    blk = (W2[128 * j:128 * j + 128, :, ky, kx][:, g]
                           * m2[j][:, None]).T                  # [s, 128co]
                    blocks.append(blk)
            packs[f"wt2m_{gi}"] = np.ascontiguousarray(
                np.concatenate(blocks, axis=1)).astype(_BF16)   # [s, 18*128]
        else:
            # block order: center tap first so the plane's compute-written
            # rows sit at partition base 0 (engine AP alignment rule)
            arr = np.zeros((9 * s, 256), np.float32)
            for b, t in enumerate(BORD):
                ky, kx = TAPS[t]
                for j in range(2):
                    arr[b * s:(b + 1) * s, j * 128:(j + 1) * 128] = \
                        (W2[128 * j:128 * j + 128, :, ky, kx][:, g]
                         * m2[j][:, None]).T
            packs[f"wt2o_{gi}"] = arr.astype(_BF16)             # [9s, 256]
    return packs


def _pack_aff(gamma1, beta1, gamma2, beta2, groups):
    G = len(groups)
    out = np.zeros((128, 2 * G + 4), np.float32)
    for gi, g in enumerate(groups):
        s = len(g)
        reps = 9 if (9 * s <= 128 and s < 128) else 1
        out[:s * reps, gi] = np.tile(gamma1[g], reps)
        out[:s * reps, G + gi] = np.tile(beta1[g], reps)
    g2 = np.asarray(gamma2, np.float32).reshape(2, 128)
    b2 = np.asarray(beta2, np.float32).reshape(2, 128)
    out[:, 2 * G + 0] = g2[0]
    out[:, 2 * G + 1] = g2[1]
    out[:, 2 * G + 2] = b2[0]
    out[:, 2 * G + 3] = b2[1]
    return out


def _build(groups, kinds):
    import concourse.bass as bass_mod
    import concourse.bacc as bacc
    import concourse.mybir as mybir
    import concourse.tile as tile

    f32 = mybir.dt.float32
    bf16 = mybir.dt.bfloat16
    AX = mybir.AxisListType
    ALU = mybir.AluOpType
    AF = mybir.ActivationFunctionType

    G = len(groups)
    sizes = [len(g) for g in groups]
    # stats-exchange column layouts.  "full" groups carry (sum, sumsq) column
    # pairs; a trailing i2c group packs its s sums at partitions [0:s] and its
    # s sumsqs at partitions [s:2s] of ONE column (2s <= 128).
    n_i2c = sum(1 for k in kinds if k == "i2c")
    SW1 = 2 * (G - n_i2c) + n_i2c        # BN1 exchange width (3 for [128,9])

    nc = bacc.Bacc("TRN2", target_bir_lowering=False, debug=False,
                   num_devices=N_CORES)

    x_d = nc.dram_tensor("x", [NL, C, H, W], f32, kind="ExternalInput")
    wt1_d = [nc.dram_tensor(f"wt1_{gi}", [128, 18 * sizes[gi]], bf16,
                            kind="ExternalInput") for gi in range(G)]
    wt2_d = []
    for gi in range(G):
        if kinds[gi] == "full":
            wt2_d.append(nc.dram_tensor(f"wt2m_{gi}", [sizes[gi], 18 * 128],
                                        bf16, kind="ExternalInput"))
        else:
            wt2_d.append(nc.dram_tensor(f"wt2o_{gi}", [9 * sizes[gi], 256],
                                        bf16, kind="ExternalInput"))
    aff_d = nc.dram_tensor("aff", [128, 2 * G + 4], f32, kind="ExternalInput")
    out_d = nc.dram_tensor("out", [NL, C, H, W], f32, kind="ExternalOutput")

    replica_groups = [list(range(N_CORES))]

    def interior(tile_ap, base, nrows):
        """[p, nrows, 56] strided view (row stride PW) starting at `base`."""
        v = tile_ap[:, base:base + nrows * PW].rearrange(
            "p (r c) -> p r c", c=PW)
        return v[:, :, 0:W]

    with tile.TileContext(nc) as tc:
        import contextlib
        with contextlib.ExitStack() as ctx:
            const = ctx.enter_context(tc.tile_pool(name="const", bufs=1))
            psum = ctx.enter_context(tc.tile_pool(name="psum", bufs=6,
                                                  space="PSUM"))
            psumb = ctx.enter_context(tc.tile_pool(name="psumb", bufs=2,
                                                   space="PSUM"))
            xst = ctx.enter_context(tc.tile_pool(name="xst", bufs=3))
            otp = ctx.enter_context(tc.tile_pool(name="otp", bufs=2))
            sqp = ctx.enter_context(tc.tile_pool(name="sqp", bufs=2))

            wt1_sb = [const.tile([128, 18 * sizes[gi]], bf16,
                                 tag=f"wt1_{gi}", name=f"wt1_{gi}")
                      for gi in range(G)]
            wt2_sb = []
            for gi in range(G):
                if kinds[gi] == "full":
                    wt2_sb.append(const.tile([sizes[gi], 18 * 128], bf16,
                                             tag=f"wt2_{gi}", name=f"wt2_{gi}"))
                else:
                    wt2_sb.append(const.tile([9 * sizes[gi], 256], bf16,
                                             tag=f"wt2_{gi}", name=f"wt2_{gi}"))
            for gi in range(G):
                nc.sync.dma_start(wt1_sb[gi][:], wt1_d[gi][:])
                nc.sync.dma_start(wt2_sb[gi][:], wt2_d[gi][:])
            aff_sb = const.tile([128, 2 * G + 4], f32, tag="aff", name="aff")
            nc.sync.dma_start(aff_sb[:], aff_d[:])

            # persistent per-image planes
            x_pad = [[const.tile([128, PLANE], bf16, tag=f"xp{j}_{n}",
                                 name=f"xp{j}_{n}")
                      for n in range(NL)] for j in range(2)]
            h1 = []                       # per group: list over images
            for gi in range(G):
                rows = 9 * sizes[gi] if kinds[gi] == "i2c" else sizes[gi]
                h1.append([const.tile([rows, PLANE], bf16, tag=f"h1{gi}_{n}",
                                      name=f"h1{gi}_{n}")
                           for n in range(NL)])
            h2 = [[const.tile([128, HW], bf16, tag=f"h2{j}_{n}",
                              name=f"h2{j}_{n}")
                   for n in range(NL)] for j in range(2)]

            # zero the non-interior positions of padded planes (i2c planes:
            # center block only; other blocks are fully DMA-overwritten with
            # shifted copies whose source pads are these zeros)
            # pad zeroing on gpsimd (idle at entry) so the DVE FIFO is
            # free for the x casts and the first matmul can issue early
            def zero_pads(t, s):
                tt = t[0:s]
                nc.gpsimd.memset(tt[:, 0:STRIP0], 0.0)
                pairs = tt[:, 2 * PW - 1:2 * PW - 1 + 56 * PW].rearrange(
                    "p (r c) -> p r c", c=PW)[:, :, 0:2]
                nc.gpsimd.memset(pairs, 0.0)
                nc.gpsimd.memset(tt[:, STRIP0 + 56 * PW:PLANE], 0.0)

            # receive tiles for the stats exchanges: pre-zeroed (before any
            # peer can arrive -- peers only send after their conv1, >200us
            # out) so the unwritten own slot contributes zero to the sum.
            rv1 = const.tile([128, 8 * SW1], f32, tag="rv1", name="rv1")
            rv2 = [const.tile([128, 16], f32, tag=f"rv2{j}", name=f"rv2{j}")
                   for j in range(2)]
            nc.gpsimd.memset(rv1[:], 0.0)
            nc.gpsimd.memset(rv2[0][:], 0.0)
            nc.gpsimd.memset(rv2[1][:], 0.0)

            for j in range(2):
                for n in range(NL):
                    zero_pads(x_pad[j][n], 128)
            for gi in range(G):
                for n in range(NL):
                    zero_pads(h1[gi][n], sizes[gi])

            # ---- cross-core stats exchange plumbing (SBUF remote DMA) ----
            # Three exchanges (BN1, BN2 half0, BN2 half1).  Each is ONE
            # 8-destination remote broadcast: core `p` lands its packed
            # [128, w] stats tile in slot p of every core's receive tile
            # (the slot offset is p*w, computed at descgen time from the
            # partition-id register).  Each arrival bumps rsem by +2, so a
            # full exchange is rsem >= 14 (7 peers; own slot stays zero).
            # Descriptors are PRE-GENERATED here; a single gpsimd
            # trigger_dma fires each when its stats tile is ready.
            rsem = [nc.alloc_semaphore(f"rst{i}") for i in range(3)]
            lsem = nc.alloc_semaphore("lst")
            _gp_prev = [None]
            deferred_waits = []

            def gp_order(bi):
                if _gp_prev[0] is not None:
                    bass_mod._add_dep_helper(bi.ins, _gp_prev[0].ins,
                                             sync=False,
                                             reason="stats-exchange order")
                _gp_prev[0] = bi
                return bi

            nc._bir_kernel_barrier_sem_replica_groups.extend(
                set(g) for g in replica_groups)

            def defer_wait(bi, sem, val):
                bi._wait_ge(sem, 0)
                deferred_waits.append((bi, sem, val))
                return bi

            # sems persist across NEFF executions: clear them as soon as all
            # cores have entered (peers send >100us later, after conv1)
            for i, s in enumerate(rsem + [lsem]):
                cl = gp_order(nc.gpsimd.sem_clear(s))
                if i == 0:
                    defer_wait(cl, nc._bir_kernel_barrier_sem,
                               nc.bir_kernel_barrier_sem_inc)

            pid = nc.gpsimd.partition_id()

            # stats tiles written at conv end; descriptors reference them now.
            # rdests slot 0 is a dummy: a core never sends to itself (XOR
            # distance 0); its own stats are added locally at unpack time.
            packed1 = const.tile([128, SW1], f32, tag="pk1", name="pk1")
            packed2 = [const.tile([128, 2], f32, tag=f"pk2{j}", name=f"pk2{j}")
                       for j in range(2)]
            peer_dests = [None] + [(0, d) for d in range(1, 8)]
            for ex, (pk, rv, w) in enumerate(((packed1, rv1, SW1),
                                              (packed2[0], rv2[0], 2),
                                              (packed2[1], rv2[1], 2))):
                dst = rv[:, 0:w]
                dst.offset = pid * w + dst.offset
                gp_order(nc.gpsimd.remote_dma_broadcast(
                    dst, pk[:], remote_sem=rsem[ex], local_sem=lsem,
                    rdests=peer_dests))

            # per-(image,chunk) (sum, sumsq) accumulator columns, filled by
            # the ACT drain copies' accum_out and a paired ACT square op
            acc1 = {(gi, sq): const.tile([sizes[gi], NL * NCHUNK], f32,
                                         tag=f"a1{gi}{sq}", name=f"a1{gi}{sq}")
                    for gi in range(G) for sq in ("s", "q")}
            acc2 = {(j, sq): const.tile([128, NL * NCHUNK], f32,
                                        tag=f"a2{j}{sq}", name=f"a2{j}{sq}")
                    for j in range(2) for sq in ("s", "q")}

            # ---- head: stream x in (all DMAs up front; staging ring 4),
            # casts emitted per image inside the conv1 loop below ----
            head_xs = []
            for n in range(NL):
                for rh in range(2):
                    for j in range(2):
                        r0 = rh * HALF_ROWS
                        xs = xst.tile([128, HALF_ELEMS], f32, tag="xs",
                                      name="xs")
                        nc.sync.dma_start(
                            xs[:],
                            x_d[n, j * 128:(j + 1) * 128, r0:r0 + HALF_ROWS, :])
                        head_xs.append((n, rh, j, xs))

            def emit_casts(n):
                for (nn, rh, j, xs) in head_xs:
                    if nn != n:
                        continue
                    r0 = rh * HALF_ROWS
                    dst = interior(x_pad[j][nn], (r0 + 1) * PW + 1, HALF_ROWS)
                    src = xs[:, :].rearrange("p (r c) -> p r c", c=W)
                    nc.vector.tensor_copy(dst, src)

            # ---- conv1: per chunk, one 18-matmul run for the full group.
            # The tiny i2c overflow group (s<=32 outputs) runs its 18 taps
            # as 4 CONCURRENT PE column-tiles (tile_position=(0,32c), ~5
            # stream passes instead of 18); the 4 partition slices of the
            # accumulated PSUM bank are then folded with DVE adds (slice
            # bases 0/32/64/96 are all legal engine AP bases) and the sum /
            # sum-of-squares statistics ride on the DVE ops' accum_out, so
            # the ACT engine carries no overflow work at all. ----
            emit_casts(0)
            emit_casts(1)
            c1_last = None
            for n in range(NL):
                if n + 2 < NL:
                    emit_casts(n + 2)
                for k in range(NCHUNK):
                    col = n * NCHUNK + k
                    base = (1 + 8 * k) * PW + 1
                    for gi in range(G):
                        s = sizes[gi]
                        if kinds[gi] == "full":
                            pt = psum.tile([s, 8 * W], f32, tag="ps",
                                           name="ps")
                            for idx, (hh, (ky, kx)) in enumerate(
                                    (hh, t) for hh in range(2) for t in TAPS):
                                dq = (ky - 1) * PW + (kx - 1)
                                off = STRIP0 + CHUNK * k + dq
                                rhs = x_pad[hh][n][:, off:off + CHUNK] \
                                    .rearrange("p (r c) -> p r c",
                                               c=PW)[:, :, 0:W]
                                nc.tensor.matmul(
                                    pt[:],
                                    wt1_sb[gi][:, idx * s:(idx + 1) * s],
                                    rhs, start=(idx == 0), stop=(idx == 17))
                            dst = interior(h1[gi][n][0:s], base, 8)
                            src = pt[:, 0:8 * W].rearrange(
                                "p (r c) -> p r c", c=W)
                            nc.scalar.activation(
                                dst, src, AF.Copy,
                                accum_out=acc1[(gi, "s")][:, col:col + 1])
                            sq = sqp.tile([128, 8 * W], f32, tag="sq",
                                          name="sq")
                            c1_last = nc.vector.scalar_tensor_tensor(
                                sq[0:s, :].rearrange("p (r c) -> p r c", c=W),
                                src, 1.0, dst, ALU.mult, ALU.mult,
                                accum_out=acc1[(gi, "q")][:, col:col + 1])
                        else:
                            p4 = psumb.tile([128, 8 * W], f32, tag="psb",
                                            name="psb")
                            for idx, (hh, (ky, kx)) in enumerate(
                                    (hh, t) for hh in range(2) for t in TAPS):
                                c4 = idx % 4
                                dq = (ky - 1) * PW + (kx - 1)
                                off = STRIP0 + CHUNK * k + dq
                                rhs = x_pad[hh][n][:, off:off + CHUNK] \
                                    .rearrange("p (r c) -> p r c",
                                               c=PW)[:, :, 0:W]
                                nc.tensor.matmul(
                                    p4[32 * c4:32 * c4 + s, 0:8 * W],
                                    wt1_sb[gi][:, idx * s:(idx + 1) * s],
                                    rhs, start=(idx < 4), stop=(idx >= 14),
                                    tile_position=(0, 32 * c4))
            # a DVE op may read at most ONE PSUM operand: drain the
                            # 4-slice bank to SBUF once (ACT), then chain
                            # adds that reuse drained PSUM slices as
                            # intermediates (slice bases 0/32/64/96 are all
                            # legal engine AP bases)
                            ysb = sqp.tile([128, 8 * W], f32, tag="sq",
                                           name="sq")
                            nc.scalar.activation(ysb[:], p4[:], AF.Copy)
                            nc.vector.tensor_tensor(
                                ysb[0:s, 0:8 * W], p4[0:s, 0:8 * W],
                                ysb[32:32 + s, 0:8 * W], ALU.add)
                            nc.vector.tensor_tensor(
                                ysb[32:32 + s, 0:8 * W],
                                p4[64:64 + s, 0:8 * W],
                                ysb[0:s, 0:8 * W], ALU.add)
                            dst = interior(h1[gi][n][0:s], base, 8)
                            nc.vector.scalar_tensor_tensor(
                                dst,
                                p4[96:96 + s, 0:8 * W].rearrange(
                                    "p (r c) -> p r c", c=W),
                                1.0,
                                ysb[32:32 + s, 0:8 * W].rearrange(
                                    "p (r c) -> p r c", c=W),
                                ALU.mult, ALU.add,
                                accum_out=acc1[(gi, "s")][:, col:col + 1])
                            c1_last = nc.vector.scalar_tensor_tensor(
                                ysb[0:s, :].rearrange("p (r c) -> p r c",
                                                      c=W),
                                dst, 1.0, dst, ALU.mult, ALU.mult,
                                accum_out=acc1[(gi, "q")][:, col:col + 1])

            # ---- BN1 stats: aggregate into the packed wire layout and fire
            # the broadcast IMMEDIATELY (before the bulk replication DMAs
            # below hit the queues) ----
            # wire layout per group: "full" -> (sum, sumsq) column pair;
            # "i2c" -> one column with sums at partitions [0:s] and sumsqs
            # at partitions [32:32+s] (engine APs must start at a partition
            # base in {0,32,64,96}, so s <= 32 rides in one column).
            stats_ready = None
            colp = 0
            for gi in range(G):
                s = sizes[gi]
                if kinds[gi] == "full":
                    nc.vector.tensor_reduce(
                        packed1[0:s, colp:colp + 1], acc1[(gi, "s")][:],
                        axis=AX.X, op=ALU.add)
                    stats_ready = nc.vector.tensor_reduce(
                        packed1[0:s, colp + 1:colp + 2], acc1[(gi, "q")][:],
                        axis=AX.X, op=ALU.add)
                    colp += 2
                else:
                    nc.vector.tensor_reduce(
                        packed1[0:s, colp:colp + 1], acc1[(gi, "s")][:],
                        axis=AX.X, op=ALU.add)
                    stats_ready = nc.vector.tensor_reduce(
                        packed1[32:32 + s, colp:colp + 1], acc1[(gi, "q")][:],
                        axis=AX.X, op=ALU.add)
                    colp += 1
            tr1 = gp_order(nc.gpsimd.trigger_dma(count=1))
            bass_mod._add_dep_helper(tr1.ins, stats_ready.ins, sync=True,
                                     reason="stats1 ready")

            # ---- wait for all 8 contributions, sum slots -> global stats ----
            gl1 = const.tile([128, SW1], f32, tag="gl1", name="gl1")
            rec1 = nc.vector.tensor_reduce(
                gl1[:], rv1[:, 0:8 * SW1].rearrange("p (s c) -> p c s", c=SW1),
                axis=AX.X, op=ALU.add)
            defer_wait(rec1, rsem[0], 14)
            bass_mod._add_dep_helper(rec1.ins, c1_last.ins, sync=True,
                                     reason="recv after conv phase")

            # unpack to the affine layout gl1x [128, 2G]: (sum, sumsq) column
            # pairs per group, adding this core's own packed stats (its rv
            # slot is the zeroed dummy).  i2c stats stay on the base rows
            # [0:s]; the BN1 apply touches only the center tap block and the
            # tap replicas are copied AFTER the apply (see conv2 loop).
            gl1x = const.tile([128, 2 * G], f32, tag="gl1x", name="gl1x")
            colp = 0
            gx = 0
            for gi in range(G):
                s = sizes[gi]
                if kinds[gi] == "full":
                    nc.vector.tensor_tensor(gl1x[0:s, gx:gx + 2],
                                            gl1[0:s, colp:colp + 2],
                                            packed1[0:s, colp:colp + 2],
                                            ALU.add)
                    colp += 2
                else:
                    nc.vector.tensor_tensor(
                        gl1x[0:s, gx:gx + 1],
                        gl1[0:s, colp:colp + 1],
                        packed1[0:s, colp:colp + 1], ALU.add)
                    nc.vector.tensor_tensor(
                        gl1x[0:s, gx + 1:gx + 2],
                        gl1[32:32 + s, colp:colp + 1],
                        packed1[32:32 + s, colp:colp + 1], ALU.add)
                    colp += 1
                gx += 2

            # ---- BN affine from global stats (DVE-only; fast rsqrt) ----
            def bn_affine(gl, w, g_col, b_col, sfx):
                """gl [128, 2w] = (sum, sumsq) pairs -> scale/bias [128, w]."""
                glv = gl[:, 0:2 * w].rearrange("p (g c) -> p g c", c=2)
                mean = const.tile([128, w], f32, tag=f"mean{sfx}",
                                  name=f"mean{sfx}")
                nc.vector.tensor_scalar_mul(mean[:], glv[:, :, 0],
                                            1.0 / COUNT)
                var = const.tile([128, w], f32, tag=f"var{sfx}",
                                 name=f"var{sfx}")
                nc.vector.tensor_tensor(var[:], mean[:], mean[:], ALU.mult)
                nc.vector.scalar_tensor_tensor(
                    var[:], glv[:, :, 1], 1.0 / COUNT, var[:],
                    ALU.mult, ALU.subtract)
                nc.vector.tensor_scalar_add(var[:], var[:], EPS)
                y = const.tile([128, w], f32, tag=f"y{sfx}", name=f"y{sfx}")
                nc.scalar.activation(y[:], var[:], AF.Sqrt)
                nc.vector.reciprocal(y[:], y[:])
                sc = const.tile([128, w], f32, tag=f"sc{sfx}", name=f"sc{sfx}")
                nc.vector.tensor_tensor(sc[:], aff_sb[:, g_col:g_col + w],
                                        y[:], ALU.mult)
                bi = const.tile([128, w], f32, tag=f"bi{sfx}", name=f"bi{sfx}")
                nc.vector.tensor_tensor(bi[:], mean[:], sc[:], ALU.mult)
                nc.vector.tensor_tensor(bi[:], aff_sb[:, b_col:b_col + w],
                                        bi[:], ALU.subtract)
                return sc, bi

            sc1, bi1 = bn_affine(gl1x, G, 0, G, "1")

            # ---- per image: BN1 apply (ACT relu, rh-major so conv2's first
            # chunks unblock quickly).  i2c groups apply only the center tap
            # block; the 8 tap-shifted replicas are DMA-copied from the
            # POST-relu center block right after (overlapping conv2's
            # full-group matmuls, which don't consume them). ----
            def emit_apply(n):
                for rh in range(2):
                    base = (1 + rh * HALF_ROWS) * PW + 1
                    for gi in range(G):
                        s = sizes[gi]
                        v = interior(h1[gi][n][0:s], base, HALF_ROWS)
                        nc.scalar.activation(
                            v, v, AF.Relu,
                            bias=bi1[0:s, gi:gi + 1],
                            scale=sc1[0:s, gi:gi + 1])

            def emit_repl(n):
                for gi in range(G):
                    if kinds[gi] != "i2c":
                        continue
                    s = sizes[gi]
                    for b, t in enumerate(BORD):
                        if b == 0:
                            continue
                        ky, kx = TAPS[t]
                        dq = (ky - 1) * PW + (kx - 1)
                        nc.sync.dma_start(
                            h1[gi][n][b * s:(b + 1) * s,
                                      STRIP0:STRIP0 + 56 * PW],
                            h1[gi][n][0:s,
                                      STRIP0 + dq:STRIP0 + 56 * PW + dq])

            # ---- conv2, output-half-major.  Half j's stats fire as soon as
            # its last chunk drains; half 0's exchange and tail overlap half
            # 1's matmuls (tail compute on gpsimd, which is idle here). ----
            sc2 = [None, None]
            bi2 = [None, None]

            def emit_conv2_chunk(n, k, j):
                col = n * NCHUNK + k
                pt = psum.tile([128, 8 * W], f32, tag="ps", name="ps")
                nmm = sum(9 if kinds[gi] == "full" else 1
                          for gi in range(G))
                idx = 0
                for gi in range(G):
                    s = sizes[gi]
                    if kinds[gi] == "full":
                        for t, (ky, kx) in enumerate(TAPS):
                            dq = (ky - 1) * PW + (kx - 1)
                            off = STRIP0 + CHUNK * k + dq
                            rhs = h1[gi][n][0:s, off:off + CHUNK] \
                                .rearrange("p (r c) -> p r c",
                                           c=PW)[:, :, 0:W]
                            nc.tensor.matmul(
                                pt[:],
                                wt2_sb[gi][:, (j * 9 + t) * 128:
                                           (j * 9 + t + 1) * 128],
                                rhs, start=(idx == 0),
                                stop=(idx == nmm - 1))
                            idx += 1
                    else:
                        off = STRIP0 + CHUNK * k
                        rhs = h1[gi][n][0:9 * s, off:off + CHUNK] \
                            .rearrange("p (r c) -> p r c",
                                       c=PW)[:, :, 0:W]
                        nc.tensor.matmul(
                            pt[:],
                            wt2_sb[gi][:, j * 128:(j + 1) * 128],
                            rhs, start=(idx == 0),
                            stop=(idx == nmm - 1))
                        idx += 1
                dst = h2[j][n][:, 8 * k * W:(8 * k + 8) * W] \
                    .rearrange("p (r c) -> p r c", c=W)
                src = pt[:, 0:8 * W].rearrange("p (r c) -> p r c", c=W)
                nc.scalar.activation(
                    dst, src, AF.Copy,
                    accum_out=acc2[(j, "s")][:, col:col + 1])
                sq = sqp.tile([128, 8 * W], f32, tag="sq", name="sq")
                return nc.vector.scalar_tensor_tensor(
                    sq[:, :].rearrange("p (r c) -> p r c", c=W),
                    src, 1.0, dst, ALU.mult, ALU.mult,
                    accum_out=acc2[(j, "q")][:, col:col + 1])

            def emit_stats2(j, c2_last):
                nc.vector.tensor_reduce(
                    packed2[j][:, 0:1], acc2[(j, "s")][:], axis=AX.X,
                    op=ALU.add)
                red = nc.vector.tensor_reduce(
                    packed2[j][:, 1:2], acc2[(j, "q")][:], axis=AX.X,
                    op=ALU.add)
                tr = gp_order(nc.gpsimd.trigger_dma(count=1))
                bass_mod._add_dep_helper(tr.ins, red.ins, sync=True,
                                         reason=f"stats2{j} ready")
                gl = const.tile([128, 2], f32, tag=f"gl2{j}", name=f"gl2{j}")
                rec = nc.vector.tensor_reduce(
                    gl[:], rv2[j][:, 0:16].rearrange("p (s c) -> p c s", c=2),
                    axis=AX.X, op=ALU.add)
                defer_wait(rec, rsem[1 + j], 14)
                bass_mod._add_dep_helper(rec.ins, c2_last.ins, sync=True,
                                         reason="recv after conv phase")
                nc.vector.tensor_tensor(gl[:], gl[:], packed2[j][:], ALU.add)
                sc2[j], bi2[j] = bn_affine(gl, 1, 2 * G + j, 2 * G + 2 + j,
                                           f"2{j}")

            def emit_tail_piece(n, rh, j, eng):
                """out[n, half j, row-half rh] = relu(sc2*h2 + bi2 + x).

                eng: 'dv2' = both elementwise ops on DVE (used for half 0,
                which overlaps half 1's matmul stream where ACT carries the
                drain traffic and DVE is idle); 'dv' = DVE stt + ACT relu
                (the final tail, where both engines are free).
                """
                r0 = rh * HALF_ROWS
                xv = interior(x_pad[j][n], (r0 + 1) * PW + 1, HALF_ROWS)
                h2v = h2[j][n][:, r0 * W:r0 * W + HALF_ELEMS].rearrange(
                    "p (r c) -> p r c", c=W)
                tb = otp.tile([128, HALF_ELEMS], bf16, tag="tb",
                              name="tb", bufs=2)
                tbv = tb[:, :].rearrange("p (r c) -> p r c", c=W)
                pool = otp if (rh + j) % 2 == 0 else xst
                ot = pool.tile([128, HALF_ELEMS], f32,
                               tag="ot" if (rh + j) % 2 == 0 else "xs",
                               name="ot")
                nc.vector.scalar_tensor_tensor(
                    tbv, h2v, sc2[j][:, 0:1], xv, ALU.mult, ALU.add)
                if eng == "dv2":
                    nc.vector.tensor_scalar(
                        ot[:], tb[:], bi2[j][:, 0:1], 0.0,
                        ALU.add, ALU.max)
                else:
                    nc.scalar.activation(ot[:], tb[:], AF.Relu,
                                         bias=bi2[j][:, 0:1], scale=1.0)
                nc.sync.dma_start(
                    out_d[n, j * 128:(j + 1) * 128, r0:r0 + HALF_ROWS, :],
                    ot[:])

            # half 0: BN1 applies + tap replication pipeline one image ahead
            # of the matmuls
            emit_apply(0)
            emit_repl(0)
            c2_last = None
            for n in range(NL):
                if n + 1 < NL:
                    emit_apply(n + 1)
                    emit_repl(n + 1)
                for k in range(NCHUNK):
                    c2_last = emit_conv2_chunk(n, k, 0)
            emit_stats2(0, c2_last)

            # half 1: interleave half 0's tail (gpsimd + DMA, both idle
            # during the matmul stream).  One image of lag keeps the
            # strict-FIFO queues from damming behind the exchange wait.
            tail0 = [(n, rh) for n in range(NL) for rh in range(2)]
            ti = 0
            for n in range(NL):
                for k in range(NCHUNK):
                    c2_last = emit_conv2_chunk(n, k, 1)
                while ti < len(tail0) and tail0[ti][0] <= n - 2:
                    tn, trh = tail0[ti]
                    emit_tail_piece(tn, trh, 0, "dv2")
                    ti += 1
            emit_stats2(1, c2_last)
            while ti < len(tail0):
                tn, trh = tail0[ti]
                emit_tail_piece(tn, trh, 0, "dv2")
                ti += 1

            # half 1's tail: DVE stt chain + ACT relu chain, pipelined
            for p, (n, rh) in enumerate((n, rh) for n in range(NL)
                                        for rh in range(2)):
                emit_tail_piece(n, rh, 1, "dv")

    # patch the reserved wait slots to their real thresholds now that
    # scheduling is done (the single-core scheduling simulator cannot
    # satisfy remote increments)
    for bi, sem, val in deferred_waits:
        patched = False
        for wv in bi.ins.sync_info.on_wait:
            if wv.id == sem.num and wv.wait_value == 0:
                wv.wait_value = val
                patched = True
                break
        assert patched, f"deferred wait not found on {bi.ins.name}"

    nc.compile()
    return nc


def kernel(x, W1, W2, gamma1, beta1, gamma2, beta2, mask1, mask2,
           _trace=False, _trace_kwargs=None):
    from concourse.bass_utils import run_bass_kernel_spmd

    mask1 = np.asarray(mask1, np.float32)
    mask2 = np.asarray(mask2, np.float32)
    beta1 = np.asarray(beta1, np.float32)
    groups, kinds = _make_plan(mask1, beta1)
    key = (tuple(len(g) for g in groups), tuple(kinds))
    if _cache.get("key") != key:
        _cache["nc"] = _build(groups, kinds)
        _cache["key"] = key
    nc = _cache["nc"]

    packs = _pack_weights(np.asarray(W1, np.float32),
                          np.asarray(W2, np.float32), mask2, groups, kinds)
    aff = _pack_aff(np.asarray(gamma1, np.float32), beta1,
                    np.asarray(gamma2, np.float32),
                    np.asarray(beta2, np.float32), groups)
    x = np.ascontiguousarray(np.asarray(x, np.float32))

    in_maps = [dict(packs, x=x[i * NL:(i + 1) * NL], aff=aff)
               for i in range(N_CORES)]
    kw = {}
    if _trace:
        kw = dict(trace=True, **(_trace_kwargs or {}))
    res = run_bass_kernel_spmd(nc, in_maps, core_ids=list(range(N_CORES)), **kw)
    out = np.concatenate([res.results[i]["out"] for i in range(N_CORES)],
                         axis=0)
    _cache["last_results"] = res
    return out
